# revision 16
# baseline (speedup 1.0000x reference)
"""Trainium2 Bass kernel for nn_Encoder (GAT message-passing encoder).

Contract: kernel(**inputs) takes the FULL unsharded inputs (as produced by
setup_inputs) and returns (node_embedding [32,512,256] f32, adj [32,14,512,512] i32).

Strategy (data-parallel over batch, 4 batches per core on 8 cores):
  * adjacency: planes 0..10 are zero except <=510 scattered ones per batch whose
    (row, col) positions are static -- only the plane index depends on shot_type.
    Planes 11..13 are batch-independent constants. Outputs are pre-zeroed by
    run_bass_kernel_spmd, so the kernel writes only: the 3 const planes (from
    SBUF-resident constants) and 512 row-scatter writes per batch via indirect
    DMA (flat row index = b*14*512 + shot_plane*512 + row, computed on host).
  * GAT: the relation-union mask is provably all-ones-minus-diagonal, so both
    GAT layers are dense softmax attention with the diagonal excluded; the
    diagonal term exp(lrelu(s_src[i]+s_dst[i])) is subtracted from numerator
    and denominator instead of masking. Scores are rank-1 (s_src[i]+s_dst[j]),
    built transposed ([j_partition, i_free]) via a PE broadcast matmul; then
    exp(lrelu(x)) = max(exp(x), exp(0.2 x)) via two ACT Exp passes + a DVE max.
    attn@h runs as probsT.T @ h_aug where h_aug carries a ones column so the
    softmax denominator falls out of the same matmul.  Raw exp is safe: scores
    stay in [-4, 3].

Implementation is RAW BASS (explicit engine streams + semaphores): the walrus
build in this environment rejects any instruction carrying more than one
embedded sync-wait, which the Tile framework's scheduler (and its closing
Drain) always produces.  Standalone wait_ge instructions have no such limit.
A small planner tracks each physical buffer's writer/readers and emits the
minimal cross-engine waits, eliding ones already covered by each engine's
high-water mark.
"""

import numpy as np
from contextlib import ExitStack

B, E = 32, 256
N = 2 * E            # 512 nodes
S = E - 1            # 255 steps
H = 4                # heads
HID = 64             # per-head feature dim
NCORES = 8
BLOC = B // NCORES   # 4 batches per core
P = 128
NT = N // P          # 4 row tiles
NPLANES = 14
W = HID + 1          # h_aug per-head width (64 feats + ones col)


# ---------------------------------------------------------------------------
# static structure (depends only on E)
# ---------------------------------------------------------------------------

def _static_adj():
    base = np.zeros((NPLANES, N, N), dtype=np.int32)
    base[13] = 1 - np.eye(N, dtype=np.int32)
    for row in range(N):
        ni = row // 2
        if row % 2 == 0:
            col = (ni + 1) * 2
            rel = 11 if ni % 2 == 0 else 12
        else:
            col = (ni + 1) * 2 + 1
            rel = 12 if ni % 2 == 0 else 11
        if col <= N - 1:
            base[rel, row, col] = 1
            base[rel, col, row] = 1
            base[13, row, col] = 0
            base[13, col, row] = 0
    s_arr = np.arange(S)
    even = s_arr % 2 == 0
    i_s = np.where(even, 2 * s_arr, 2 * s_arr + 1)
    j_s = np.where(even, 2 * (s_arr + 1) + 1, 2 * (s_arr + 1))
    const13 = base[13].copy()
    const13[i_s, j_s] = 0
    const13[j_s, i_s] = 0
    const3 = np.stack([base[11], base[12], const13])          # [3,N,N]

    rows = np.concatenate([i_s, j_s])                          # 510 scatter triples
    cols = np.concatenate([j_s, i_s])
    svec = np.concatenate([s_arr, s_arr])
    sib = np.full(510, -1, np.int64)
    byrow = {}
    for k, r in enumerate(rows):
        byrow.setdefault(int(r), []).append(k)
    for _, ks in byrow.items():
        if len(ks) == 2:
            sib[ks[0]], sib[ks[1]] = ks[1], ks[0]
    M1 = np.zeros((N, N), np.int32)
    M2 = np.zeros((N, N), np.int32)
    trip_row = np.zeros(N, np.int64)
    trip_s = np.zeros(N, np.int64)
    trip_sib_s = np.full(N, -1, np.int64)
    for slot in range(N):                                      # pad 510 -> 512 with dups of 0
        k = slot if slot < 510 else 0
        M1[slot, cols[k]] = 1
        trip_row[slot] = rows[k]
        trip_s[slot] = svec[k]
        if sib[k] >= 0:
            M2[slot, cols[sib[k]]] = 1
            trip_sib_s[slot] = svec[sib[k]]
    return const3, M1, M2, trip_row, trip_s, trip_sib_s


_CONST3, _M1, _M2, _TRIP_ROW, _TRIP_S, _TRIP_SIB_S = _static_adj()


# ---------------------------------------------------------------------------
# tiny raw-bass planner: tracks buffer deps, emits minimal standalone waits
# ---------------------------------------------------------------------------

class _Buf:
    __slots__ = ("writer", "readers")

    def __init__(self):
        self.writer = None        # (eng_name, sem_handle, ticket)
        self.readers = []


class _Plan:
    ENGINES = ("SP", "PE", "DVE", "ACT", "GP")

    DRAIN_ENGINES = ("DVE", "ACT")

    def __init__(self):
        self.ops = {e: [] for e in self.ENGINES}
        self.hiwater = {e: {} for e in self.ENGINES}
        self.sem_count = {}
        self.pending = {e: [] for e in self.ENGINES}
        self.dma_sems = set()      # ids of DMA-completion sems (OoO completion)
        self.op_idx = {e: 0 for e in self.ENGINES}
        self.drained_idx = {e: -1 for e in self.ENGINES}
        self.n_waits = 0
        self.n_drains = 0

    def _want(self, eng, waits, dep):
        dep_eng, sem, ticket = dep[0], dep[1], dep[2]
        if dep_eng == eng:
            return
        assert sem is not None and ticket is not None, \
            f"cross-engine dep from {dep_eng} to {eng} without a semaphore ticket"
        if id(sem) in self.dma_sems:
            # DMA completions are unordered: wait for every completion
            # issued so far instead of this specific one
            ticket = self.sem_count[id(sem)]
        key = id(sem)
        if self.hiwater[eng].get(key, 0) >= ticket:
            return
        prev = waits.get(key)
        waits[key] = (max(prev[0], ticket) if prev else ticket, sem)

    def op(self, eng, fn, reads=(), writes=(), sem=None, inc=0):
        waits = {}
        need_drain = False
        deps = []
        for b in reads:
            if b.writer is not None:
                deps.append(b.writer)
        for b in writes:
            if b.writer is not None:
                deps.append(b.writer)                  # WAW
            deps.extend(b.readers)                     # WAR
        for d in deps:
            if d[0] == eng and eng in self.DRAIN_ENGINES \
                    and d[3] > self.drained_idx[eng]:
                need_drain = True
            self._want(eng, waits, d)
        if need_drain:
            # same-engine RAW/WAR: pipeline writes only become visible to
            # later ops on this engine after a drain
            self.ops[eng].append(([], lambda e: e.drain(), None, 0))
            self.drained_idx[eng] = self.op_idx[eng]
            self.op_idx[eng] += 1
            self.n_drains += 1
        wait_list = []
        hw = self.hiwater[eng]
        for key, (ticket, semh) in waits.items():
            wait_list.append((semh, ticket))
            hw[key] = max(hw.get(key, 0), ticket)
            self.n_waits += 1
        ticket = None
        if sem is not None and inc:
            self.sem_count[id(sem)] = self.sem_count.get(id(sem), 0) + inc
            ticket = self.sem_count[id(sem)]
        self.ops[eng].append((wait_list, fn, sem, inc))
        tok = [eng, sem, ticket, self.op_idx[eng]]
        self.op_idx[eng] += 1
        if sem is not None and inc:
            # in-order engines: this inc also orders all earlier un-inc'd ops
            for t in self.pending[eng]:
                t[1], t[2] = sem, ticket
            self.pending[eng].clear()
        else:
            self.pending[eng].append(tok)
        for b in writes:
            b.writer = tok
            b.readers = []
        for b in reads:
            b.readers.append(tok)
        return ticket

    def replay(self, eng, proxy):
        for wait_list, fn, sem, inc in self.ops[eng]:
            for semh, ticket in wait_list:
                proxy.wait_ge(semh, ticket)
            inst = fn(proxy)
            if sem is not None and inc:
                inst.then_inc(sem, inc)


# ---------------------------------------------------------------------------
# bass kernel build (one program, SPMD over 8 cores; per-core inputs differ)
# ---------------------------------------------------------------------------

def _build_nc():
    import concourse.bass as bass
    from concourse import mybir

    dt = mybir.dt
    AF = mybir.ActivationFunctionType
    ALU = mybir.AluOpType

    nc = bass.Bass()

    coordT = nc.dram_tensor("coordT", [2, BLOC * N], dt.float32, kind="ExternalInput")
    pvecT = nc.dram_tensor("pvecT", [HID, BLOC * 2], dt.float32, kind="ExternalInput")
    scat_idx = nc.dram_tensor("scat_idx", [P, BLOC * NT], dt.int32, kind="ExternalInput")
    scat_eq = nc.dram_tensor("scat_eq", [P, BLOC * NT], dt.float32, kind="ExternalInput")
    wc = nc.dram_tensor("wc", [2, 32], dt.float32, kind="ExternalInput")
    bc = nc.dram_tensor("bc", [32, 1], dt.float32, kind="ExternalInput")
    wmc = nc.dram_tensor("wmc", [32, HID], dt.float32, kind="ExternalInput")
    lin1 = nc.dram_tensor("lin1", [HID, H * HID], dt.float32, kind="ExternalInput")
    lin2 = nc.dram_tensor("lin2", [H * HID, H * HID], dt.float32, kind="ExternalInput")
    wa1s = nc.dram_tensor("wa1s", [HID, H], dt.float32, kind="ExternalInput")
    wa1d = nc.dram_tensor("wa1d", [HID, H], dt.float32, kind="ExternalInput")
    wa2s = nc.dram_tensor("wa2s", [H * HID, H], dt.float32, kind="ExternalInput")
    wa2d = nc.dram_tensor("wa2d", [H * HID, H], dt.float32, kind="ExternalInput")

    out_emb = nc.dram_tensor("out_emb", [BLOC, N, H * HID], dt.float32, kind="ExternalOutput")
    out_adj = nc.dram_tensor("out_adj", [BLOC, NPLANES, N, N], dt.int32, kind="ExternalOutput")
    adj_rows = out_adj[:].rearrange("b k r c -> (b k r) c")

    const3_c = nc.inline_tensor(_CONST3, name="const3")
    m1_c = nc.inline_tensor(_M1.astype(np.float32), name="m1")
    m2_c = nc.inline_tensor(_M2.astype(np.float32), name="m2")
    ident_c = nc.inline_tensor(np.eye(P, dtype=np.float32), name="ident")

    pl = _Plan()
    bufs = {}

    def Bf(*key):
        k = tuple(key)
        if k not in bufs:
            bufs[k] = _Buf()
        return bufs[k]

    with ExitStack() as es:
        _n = [0]

        def sbuf(shape, dtype=dt.float32):
            _n[0] += 1
            return es.enter_context(nc.sbuf_tensor(f"sb{_n[0]}", shape, dtype))

        LDS = es.enter_context(nc.semaphore("lds"))
        LDB = es.enter_context(nc.semaphore("ldb"))
        OUT = es.enter_context(nc.semaphore("outs"))
        SCT = es.enter_context(nc.semaphore("sct"))
        SPE = es.enter_context(nc.semaphore("spe"))
        SDV = es.enter_context(nc.semaphore("sdv"))
        SAC = es.enter_context(nc.semaphore("sac"))
        for _s in (LDS, LDB, OUT, SCT):
            pl.dma_sems.add(id(_s))
        pl.sem_count[id(LDS)] = 0
        pl.sem_count[id(LDB)] = 0
        pl.sem_count[id(OUT)] = 0
        pl.sem_count[id(SCT)] = 0

        PSUM = [es.enter_context(nc.psum_tensor(f"pb{i}", [P, N], dt.float32))
                for i in range(8)]
        SC = PSUM[0:2]    # score broadcast [P,N]
        AT = PSUM[2:4]    # attention accumulators [P,W]
        HB = PSUM[4:6]    # h matmuls / ct / xT / transposes
        SM = PSUM[6:8]    # sdc pairs [P,8] / ssrc rows [1,N]

        const3_sb = sbuf([P, 3, NT, N], dt.int32)
        m1_sb = sbuf([P, NT, N])
        m2_sb = sbuf([P, NT, N])
        ident_sb = sbuf([P, P])
        idx_sb = sbuf([P, BLOC * NT], dt.int32)
        eq_sb = sbuf([P, BLOC * NT])
        wc_sb = sbuf([2, 32])
        bc_sb = sbuf([32, 1])
        wmc_sb = sbuf([32, HID])
        lin1_sb = sbuf([HID, H * HID])
        lin2_sb = sbuf([P, 2, H * HID])
        wa1s_sb = sbuf([HID, H])
        wa1d_sb = sbuf([HID, H])
        wa2s_sb = sbuf([P, 2, H])
        wa2d_sb = sbuf([P, 2, H])
        coord_sb = sbuf([2, BLOC * N])
        pvec_sb = sbuf([HID, BLOC * 2])
        ones1_sb = sbuf([1, P])

        ct_sb = [sbuf([32, N]) for _ in range(2)]
        xT_sb = [sbuf([HID, N]) for _ in range(2)]
        haug_sb = [[sbuf([P, H * W]) for _ in range(NT)] for _ in range(2)]
        sdc_sb = [[sbuf([P, H]) for _ in range(NT)] for _ in range(2)]
        sdc02_sb = [[sbuf([P, H]) for _ in range(NT)] for _ in range(2)]
        sdg_sb = [[sbuf([P, H]) for _ in range(NT)] for _ in range(2)]
        ede_sb = [[sbuf([P, 2 * H]) for _ in range(NT)] for _ in range(2)]
        ed_sb = [[sbuf([P, H]) for _ in range(NT)] for _ in range(2)]
        ssrc_sb = [[sbuf([1, N]) for _ in range(H)] for _ in range(2)]
        e1_sb = [sbuf([P, N]) for _ in range(2)]
        e2_sb = [sbuf([P, N]) for _ in range(2)]
        probs_sb = [[sbuf([P, N]) for _ in range(NT)] for _ in range(2)]
        x1_sb = [[sbuf([P, H * HID]) for _ in range(NT)] for _ in range(2)]
        x1T_sb = [[sbuf([P, N]) for _ in range(2)] for _ in range(2)]
        o_sb = [[sbuf([P, H * HID]) for _ in range(NT)] for _ in range(2)]
        cont_f_sb = [sbuf([P, N]) for _ in range(2)]
        cont_sb = [sbuf([P, N], dt.int32) for _ in range(2)]
        tmp_sb = [sbuf([P, HID]) for _ in range(2)]
        num_sb = [sbuf([P, HID]) for _ in range(2)]
        den_sb = [sbuf([P, 2]) for _ in range(2)]
        rden_sb = [sbuf([P, 2]) for _ in range(2)]

        # ---- initial loads (SP engine) ----
        small_loads = [
            (idx_sb[:], scat_idx[:], "idx"),
            (eq_sb[:], scat_eq[:], "eq"),
            (wc_sb[:], wc[:], "wc"),
            (bc_sb[:], bc[:], "bc"),
            (wmc_sb[:], wmc[:], "wmc"),
            (lin1_sb[:], lin1[:], "lin1"),
            (lin2_sb[:], lin2[:].rearrange("(kc p) f -> p kc f", p=P), "lin2"),
            (wa1s_sb[:], wa1s[:], "wa1s"),
            (wa1d_sb[:], wa1d[:], "wa1d"),
            (wa2s_sb[:], wa2s[:].rearrange("(kc p) h -> p kc h", p=P), "wa2s"),
            (wa2d_sb[:], wa2d[:].rearrange("(kc p) h -> p kc h", p=P), "wa2d"),
            (coord_sb[:], coordT[:], "coord"),
            (pvec_sb[:], pvecT[:], "pvec"),
            (ident_sb[:], ident_c[:], "ident"),
        ]
        for dst, src, nm in small_loads:
            pl.op("SP", lambda e, d=dst, s=src: e.dma_start(out=d, in_=s),
                  writes=[Bf(nm)], sem=LDS, inc=16)
        n_small = len(small_loads) * 16
        big_loads = [
            (const3_sb[:], const3_c[:].rearrange("k (g p) c -> p k g c", p=P), "const3"),
            (m1_sb[:], m1_c[:].rearrange("(g p) c -> p g c", p=P), "m1"),
            (m2_sb[:], m2_c[:].rearrange("(g p) c -> p g c", p=P), "m2"),
        ]
        for dst, src, nm in big_loads:
            pl.op("SP", lambda e, d=dst, s=src: e.dma_start(out=d, in_=s),
                  writes=[Bf(nm)], sem=LDB, inc=16)
        n_big = len(big_loads) * 16
        # HWDGE queues complete out of order: every consumer waits the group total
        for _, _, nm in small_loads:
            Bf(nm).writer = ("SP-DMA", LDS, n_small)
        for _, _, nm in big_loads:
            Bf(nm).writer = ("SP-DMA", LDB, n_big)

        pl.op("DVE", lambda e: e.memset(ones1_sb[:], 1.0),
              writes=[Bf("ones1")], sem=SDV, inc=1)

        out_dma_count = [0]

        # ------------------------------------------------------------------
        def layer_prep(p, xaps, xkeys, kchunks, linw, was, wad, lin_key,
                       was_key, wad_key):
            """h_aug + score-vector prep for one layer (batch parity p)."""
            for t in range(NT):
                hb = HB[t % 2]
                for kc in range(kchunks):
                    pl.op("PE", lambda e, hb=hb, t=t, kc=kc: e.matmul(
                              out=hb[:, :H * HID],
                              lhsT=xaps[kc][:, t * P:(t + 1) * P], rhs=linw(kc),
                              start=(kc == 0), stop=(kc == kchunks - 1)),
                          reads=[Bf(*xkeys[kc]), Bf(lin_key)],
                          writes=[Bf("HB", t % 2)],
                          sem=SPE, inc=(1 if kc == kchunks - 1 else 0))
                aug = haug_sb[p][t]
                pl.op("DVE", lambda e, aug=aug, hb=hb: e.tensor_copy(
                          aug[:].rearrange("p (h w) -> p h w", h=H)[:, :, :HID],
                          hb[:, :H * HID].rearrange("p (h f) -> p h f", h=H)),
                      reads=[Bf("HB", t % 2)],
                      writes=[Bf("haug", p, t)], sem=SDV, inc=0)
                pl.op("DVE", lambda e, aug=aug: e.memset(
                          aug[:].rearrange("p (h w) -> p h w", h=H)[:, :, HID:W], 1.0),
                      writes=[Bf("haug", p, t)], sem=SDV, inc=1)

                sm = SM[t % 2]
                for kc in range(kchunks):
                    pl.op("PE", lambda e, sm=sm, t=t, kc=kc: e.matmul(
                              out=sm[:, 0:H], lhsT=xaps[kc][:, t * P:(t + 1) * P],
                              rhs=wad(kc), start=(kc == 0), stop=(kc == kchunks - 1)),
                          reads=[Bf(*xkeys[kc]), Bf(wad_key)],
                          writes=[Bf("SM", t % 2)], sem=SPE, inc=0)
                for kc in range(kchunks):
                    pl.op("PE", lambda e, sm=sm, t=t, kc=kc: e.matmul(
                              out=sm[:, H:2 * H], lhsT=xaps[kc][:, t * P:(t + 1) * P],
                              rhs=was(kc), start=(kc == 0), stop=(kc == kchunks - 1)),
                          reads=[Bf(*xkeys[kc]), Bf(was_key)],
                          writes=[Bf("SM", t % 2)],
                          sem=SPE, inc=(1 if kc == kchunks - 1 else 0))
                sdc, sdc02 = sdc_sb[p][t], sdc02_sb[p][t]
                sdg, ede, ed = sdg_sb[p][t], ede_sb[p][t], ed_sb[p][t]
                pl.op("DVE", lambda e, sdc=sdc, sm=sm: e.tensor_copy(sdc[:], sm[:, 0:H]),
                      reads=[Bf("SM", t % 2)],
                      writes=[Bf("sdc", p, t)], sem=SDV, inc=1)
                pl.op("DVE", lambda e, sdc02=sdc02, sm=sm: e.tensor_scalar_mul(
                          sdc02[:], sm[:, 0:H], 0.2),
                      reads=[Bf("SM", t % 2)],
                      writes=[Bf("sdc02", p, t)], sem=SDV, inc=1)
                pl.op("DVE", lambda e, sdg=sdg, sdc=sdc, sm=sm: e.tensor_tensor(
                          out=sdg[:], in0=sdc[:], in1=sm[:, H:2 * H], op=ALU.add),
                      reads=[Bf("sdc", p, t), Bf("SM", t % 2)],
                      writes=[Bf("sdg", p, t)], sem=SDV, inc=1)
                pl.op("ACT", lambda e, ede=ede, sdg=sdg: e.activation(
                          out=ede[:, 0:H], in_=sdg[:], func=AF.Exp),
                      reads=[Bf("sdg", p, t)],
                      writes=[Bf("ede", p, t)], sem=SAC, inc=0)
                pl.op("ACT", lambda e, ede=ede, sdg=sdg: e.activation(
                          out=ede[:, H:2 * H], in_=sdg[:], func=AF.Exp, scale=0.2),
                      reads=[Bf("sdg", p, t)],
                      writes=[Bf("ede", p, t)], sem=SAC, inc=1)
                pl.op("DVE", lambda e, ed=ed, ede=ede: e.tensor_tensor(
                          out=ed[:], in0=ede[:, 0:H], in1=ede[:, H:2 * H], op=ALU.max),
                      reads=[Bf("ede", p, t)],
                      writes=[Bf("ed", p, t)], sem=SDV, inc=1)
            for h in range(H):
                sm = SM[(NT + h) % 2]
                for kc in range(kchunks):
                    pl.op("PE", lambda e, sm=sm, h=h, kc=kc: e.matmul(
                              out=sm[:1, :], lhsT=was(kc, h), rhs=xaps[kc][:],
                              start=(kc == 0), stop=(kc == kchunks - 1)),
                          reads=[Bf(*xkeys[kc]), Bf(was_key)],
                          writes=[Bf("SM", (NT + h) % 2)],
                          sem=SPE, inc=(1 if kc == kchunks - 1 else 0))
                srow = ssrc_sb[p][h]
                pl.op("DVE", lambda e, srow=srow, sm=sm: e.tensor_copy(srow[:], sm[:1, :]),
                      reads=[Bf("SM", (NT + h) % 2)],
                      writes=[Bf("ssrc", p, h)], sem=SDV, inc=1)

        # ------------------------------------------------------------------
        def attention(p, outk, outs, relu_out):
            """One GAT attention layer for parity p writing into outs tiles."""
            for h in range(H):
                for jt in range(NT):
                    sc = SC[jt % 2]
                    pl.op("PE", lambda e, sc=sc, h=h, p=p: e.matmul(
                              out=sc[:], lhsT=ones1_sb[:], rhs=ssrc_sb[p][h][:],
                              start=True, stop=True),
                          reads=[Bf("ones1"), Bf("ssrc", p, h)],
                          writes=[Bf("SC", jt % 2)], sem=SPE, inc=1)
                    e1, e2 = e1_sb[jt % 2], e2_sb[jt % 2]
                    pl.op("ACT", lambda e, sc=sc, e1=e1, jt=jt, h=h, p=p: e.activation(
                              out=e1[:], in_=sc[:], func=AF.Exp,
                              bias=sdc_sb[p][jt][:, h:h + 1]),
                          reads=[Bf("SC", jt % 2), Bf("sdc", p, jt)],
                          writes=[Bf("e1", jt % 2)], sem=SAC, inc=1)
                    pl.op("ACT", lambda e, sc=sc, e2=e2, jt=jt, h=h, p=p: e.activation(
                              out=e2[:], in_=sc[:], func=AF.Exp,
                              bias=sdc02_sb[p][jt][:, h:h + 1], scale=0.2),
                          reads=[Bf("SC", jt % 2), Bf("sdc02", p, jt)],
                          writes=[Bf("e2", jt % 2)], sem=SAC, inc=1)
                    pt = probs_sb[p][jt]
                    pl.op("DVE", lambda e, pt=pt, e1=e1, e2=e2: e.tensor_tensor(
                              out=pt[:], in0=e1[:], in1=e2[:], op=ALU.max),
                          reads=[Bf("e1", jt % 2), Bf("e2", jt % 2)],
                          writes=[Bf("probs", p, jt)], sem=SDV, inc=1)
                for it in range(NT):
                    at = AT[it % 2]
                    for jt in range(NT):
                        pl.op("PE", lambda e, at=at, jt=jt, it=it, h=h, p=p: e.matmul(
                                  out=at[:, :W],
                                  lhsT=probs_sb[p][jt][:, it * P:(it + 1) * P],
                                  rhs=haug_sb[p][jt][:, h * W:(h + 1) * W],
                                  start=(jt == 0), stop=(jt == NT - 1)),
                              reads=[Bf("probs", p, jt), Bf("haug", p, jt)],
                              writes=[Bf("AT", it % 2)],
                              sem=SPE, inc=(1 if jt == NT - 1 else 0))
                    tmp, numb = tmp_sb[it % 2], num_sb[it % 2]
                    den, rden = den_sb[it % 2], rden_sb[it % 2]
                    pl.op("DVE", lambda e, tmp=tmp, it=it, h=h, p=p: e.tensor_scalar_mul(
                              tmp[:], haug_sb[p][it][:, h * W:h * W + HID],
                              ed_sb[p][it][:, h:h + 1]),
                          reads=[Bf("haug", p, it), Bf("ed", p, it)],
                          writes=[Bf("tmp", it % 2)], sem=SDV, inc=0)
                    pl.op("DVE", lambda e, numb=numb, tmp=tmp, at=at: e.tensor_tensor(
                              out=numb[:], in0=at[:, :HID], in1=tmp[:], op=ALU.subtract),
                          reads=[Bf("AT", it % 2), Bf("tmp", it % 2)],
                          writes=[Bf("num", it % 2)], sem=SDV, inc=0)
                    pl.op("DVE", lambda e, den=den, at=at, it=it, h=h, p=p: e.tensor_tensor(
                              out=den[:, :1], in0=at[:, HID:W], in1=ed_sb[p][it][:, h:h + 1],
                              op=ALU.subtract),
                          reads=[Bf("AT", it % 2), Bf("ed", p, it)],
                          writes=[Bf("den", it % 2)], sem=SDV, inc=1)
                    pl.op("DVE", lambda e, rden=rden, den=den: e.reciprocal(
                              rden[:, :1], den[:, :1]),
                          reads=[Bf("den", it % 2)],
                          writes=[Bf("rden", it % 2)], sem=SDV, inc=0)
                    dst = outs[it]
                    if relu_out:
                        pl.op("DVE", lambda e, dst=dst, numb=numb, rden=rden, h=h:
                                  e.tensor_scalar(
                                      out=dst[:, h * HID:(h + 1) * HID], in0=numb[:],
                                      scalar1=rden[:, :1], scalar2=0.0,
                                      op0=ALU.mult, op1=ALU.max),
                              reads=[Bf("num", it % 2), Bf("rden", it % 2)],
                              writes=[Bf(outk, p, it, h)], sem=SDV, inc=1)
                    else:
                        pl.op("DVE", lambda e, dst=dst, numb=numb, rden=rden, h=h:
                                  e.tensor_scalar_mul(
                                      dst[:, h * HID:(h + 1) * HID], numb[:],
                                      rden[:, :1]),
                              reads=[Bf("num", it % 2), Bf("rden", it % 2)],
                              writes=[Bf(outk, p, it, h)], sem=SDV, inc=1)

        # ------------------------------------------------------------------
        # adjacency const planes: no compute deps, issue all up front
        for b in range(BLOC):
            for k in range(3):
                pl.op("SP", lambda e, b=b, k=k: e.dma_start(
                          out=out_adj[b, 11 + k].rearrange("(g p) c -> p g c", p=P),
                          in_=const3_sb[:, k]),
                      reads=[Bf("const3")], sem=OUT, inc=16)
                out_dma_count[0] += 1

        for b in range(BLOC):
            p = b % 2

            # ---------------- adjacency scatter ----------------
            for t in range(NT):
                col = b * NT + t
                cf, ci = cont_f_sb[t % 2], cont_sb[t % 2]
                pl.op("DVE", lambda e, cf=cf, t=t, col=col: e.tensor_tensor(
                          out=cf[:], in0=m2_sb[:, t],
                          in1=eq_sb[:, col:col + 1].to_broadcast([P, N]), op=ALU.mult),
                      reads=[Bf("m2"), Bf("eq")],
                      writes=[Bf("cf", t % 2)], sem=SDV, inc=0)
                pl.op("DVE", lambda e, cf=cf, t=t: e.tensor_tensor(
                          out=cf[:], in0=cf[:], in1=m1_sb[:, t], op=ALU.add),
                      reads=[Bf("m1"), Bf("cf", t % 2)],
                      writes=[Bf("cf", t % 2)], sem=SDV, inc=0)
                pl.op("DVE", lambda e, ci=ci, cf=cf: e.tensor_copy(ci[:], cf[:]),
                      reads=[Bf("cf", t % 2)],
                      writes=[Bf("ci", t % 2)], sem=SDV, inc=1)
                pl.op("GP", lambda e, ci=ci, col=col: e.indirect_dma_start(
                          out=adj_rows,
                          out_offset=bass.IndirectOffsetOnAxis(
                              ap=idx_sb[:, col:col + 1], axis=0),
                          in_=ci[:], in_offset=None),
                      reads=[Bf("ci", t % 2), Bf("idx")], sem=SCT, inc=16)

            # ---------------- encoder input x ----------------
            ctp = HB[0]
            pl.op("PE", lambda e, ctp=ctp, b=b: e.matmul(
                      out=ctp[:32, :], lhsT=wc_sb[:],
                      rhs=coord_sb[:, b * N:(b + 1) * N], start=True, stop=True),
                  reads=[Bf("wc"), Bf("coord")], writes=[Bf("HB", 0)], sem=SPE, inc=1)
            ct = ct_sb[p]
            pl.op("ACT", lambda e, ct=ct, ctp=ctp: e.activation(
                      out=ct[:], in_=ctp[:32, :], func=AF.Relu, bias=bc_sb[:, :1]),
                  reads=[Bf("HB", 0), Bf("bc")], writes=[Bf("ct", p)], sem=SAC, inc=1)
            xtp = HB[1]
            pl.op("PE", lambda e, xtp=xtp, ct=ct: e.matmul(
                      out=xtp[:HID, :], lhsT=wmc_sb[:], rhs=ct[:], start=True, stop=True),
                  reads=[Bf("wmc"), Bf("ct", p)], writes=[Bf("HB", 1)], sem=SPE, inc=1)
            xt = xT_sb[p]
            xps2 = xtp[:HID, :].rearrange("f (n two) -> f n two", two=2)
            xsb2 = xt[:].rearrange("f (n two) -> f n two", two=2)
            pl.op("DVE", lambda e, xsb2=xsb2, xps2=xps2, b=b: e.tensor_tensor(
                      out=xsb2[:, :, 0], in0=xps2[:, :, 0],
                      in1=pvec_sb[:, 2 * b:2 * b + 1].to_broadcast([HID, E]), op=ALU.add),
                  reads=[Bf("HB", 1), Bf("pvec")], writes=[Bf("xt", p)], sem=SDV, inc=0)
            pl.op("DVE", lambda e, xsb2=xsb2, xps2=xps2, b=b: e.tensor_tensor(
                      out=xsb2[:, :, 1], in0=xps2[:, :, 1],
                      in1=pvec_sb[:, 2 * b + 1:2 * b + 2].to_broadcast([HID, E]), op=ALU.add),
                  reads=[Bf("HB", 1), Bf("pvec")], writes=[Bf("xt", p)], sem=SDV, inc=1)

            # ---------------- layer 1 ----------------
            layer_prep(p, [xt[:]], [("xt", p)], 1,
                       lambda kc: lin1_sb[:],
                       lambda kc, h=None: wa1s_sb[:] if h is None else wa1s_sb[:, h:h + 1],
                       lambda kc: wa1d_sb[:],
                       "lin1", "wa1s", "wa1d")
            attention(p, "x1c", x1_sb[p], True)

            # ---------------- transpose x1 -> x1T ----------------
            for it in range(NT):
                for fb in range(2):
                    hb = HB[fb]
                    pl.op("PE", lambda e, hb=hb, it=it, fb=fb, p=p: e.transpose(
                              out=hb[:, :P], in_=x1_sb[p][it][:, fb * P:(fb + 1) * P],
                              identity=ident_sb[:]),
                          reads=[Bf("x1c", p, it, fb * 2), Bf("x1c", p, it, fb * 2 + 1),
                                 Bf("ident")],
                          writes=[Bf("HB", fb)], sem=SPE, inc=1)
                    pl.op("DVE", lambda e, hb=hb, it=it, fb=fb, p=p: e.tensor_copy(
                              x1T_sb[p][fb][:, it * P:(it + 1) * P], hb[:, :P]),
                          reads=[Bf("HB", fb)],
                          writes=[Bf("x1T", p, fb)], sem=SDV, inc=1)

            # ---------------- layer 2 ----------------
            layer_prep(p, [x1T_sb[p][0][:], x1T_sb[p][1][:]],
                       [("x1T", p, 0), ("x1T", p, 1)], 2,
                       lambda kc: lin2_sb[:, kc],
                       lambda kc, h=None: wa2s_sb[:, kc] if h is None else wa2s_sb[:, kc, h:h + 1],
                       lambda kc: wa2d_sb[:, kc],
                       "lin2", "wa2s", "wa2d")
            attention(p, "oc", o_sb[p], False)

            # ---------------- write node embedding ----------------
            for it in range(NT):
                pl.op("SP", lambda e, b=b, it=it, p=p: e.dma_start(
                          out=out_emb[b, it * P:(it + 1) * P, :], in_=o_sb[p][it][:]),
                      reads=[Bf("oc", p, it, 0), Bf("oc", p, it, 1),
                             Bf("oc", p, it, 2), Bf("oc", p, it, 3)],
                      sem=OUT, inc=16)
                out_dma_count[0] += 1

        pl.ops["SP"].append(
            ([(OUT, out_dma_count[0] * 16)], lambda e: e.nop(), None, 0))
        pl.ops["GP"].append(
            ([(SCT, BLOC * NT * 16)], lambda e: e.nop(), None, 0))

        with nc.Block() as block:
            @block.sync
            def _(eng):
                pl.replay("SP", eng)

            @block.tensor
            def _(eng):
                pl.replay("PE", eng)

            @block.vector
            def _(eng):
                pl.replay("DVE", eng)

            @block.scalar
            def _(eng):
                pl.replay("ACT", eng)

            @block.gpsimd
            def _(eng):
                pl.replay("GP", eng)

    return nc


_NC_CACHE = None


def _get_nc():
    global _NC_CACHE
    if _NC_CACHE is None:
        _NC_CACHE = _build_nc()
    return _NC_CACHE


# ---------------------------------------------------------------------------
# host-side sharding / input prep
# ---------------------------------------------------------------------------

def make_in_maps(inputs):
    emb_table = np.asarray(inputs["emb_table"], np.float32)
    Wc = np.asarray(inputs["Wc"], np.float32)
    bcv = np.asarray(inputs["bc"], np.float32)
    Wm = np.asarray(inputs["Wm"], np.float32)
    bm = np.asarray(inputs["bm"], np.float32)
    lin1_W = np.asarray(inputs["lin1_W"], np.float32)
    att1 = np.asarray(inputs["att1"], np.float32)
    lin2_W = np.asarray(inputs["lin2_W"], np.float32)
    att2 = np.asarray(inputs["att2"], np.float32)
    player = np.asarray(inputs["player"]).astype(np.int64)
    st = np.asarray(inputs["shot_type"]).astype(np.int64)
    Ax = np.asarray(inputs["player_A_x"], np.float32)
    Ay = np.asarray(inputs["player_A_y"], np.float32)
    Bx = np.asarray(inputs["player_B_x"], np.float32)
    By = np.asarray(inputs["player_B_y"], np.float32)

    pvec = (emb_table[player] @ Wm[32:] + bm).astype(np.float32)   # [B,2,HID]
    coordT = np.zeros((B, 2, N), np.float32)
    coordT[:, 0, 0::2] = Ax
    coordT[:, 1, 0::2] = Ay
    coordT[:, 0, 1::2] = Bx
    coordT[:, 1, 1::2] = By
    wa1s = np.einsum("khf,hf->kh", lin1_W.reshape(HID, H, HID), att1[:, :HID])
    wa1d = np.einsum("khf,hf->kh", lin1_W.reshape(HID, H, HID), att1[:, HID:])
    wa2s = np.einsum("khf,hf->kh", lin2_W.reshape(H * HID, H, HID), att2[:, :HID])
    wa2d = np.einsum("khf,hf->kh", lin2_W.reshape(H * HID, H, HID), att2[:, HID:])

    st_trip = st[:, _TRIP_S]                                       # [B, 512]
    sib_st = np.where(_TRIP_SIB_S >= 0, st[:, np.maximum(_TRIP_SIB_S, 0)], -1)
    eq = ((sib_st >= 0) & (sib_st == st_trip)).astype(np.float32)  # [B, 512]
    base_rows = (np.arange(B)[:, None] % BLOC) * (NPLANES * N)
    idx = (base_rows + st_trip * N + _TRIP_ROW[None, :]).astype(np.int32)

    shared = dict(
        wc=np.ascontiguousarray(Wc),
        bc=np.ascontiguousarray(bcv.reshape(32, 1)),
        wmc=np.ascontiguousarray(Wm[:32]),
        lin1=np.ascontiguousarray(lin1_W),
        lin2=np.ascontiguousarray(lin2_W),
        wa1s=np.ascontiguousarray(wa1s.astype(np.float32)),
        wa1d=np.ascontiguousarray(wa1d.astype(np.float32)),
        wa2s=np.ascontiguousarray(wa2s.astype(np.float32)),
        wa2d=np.ascontiguousarray(wa2d.astype(np.float32)),
    )
    in_maps = []
    for c in range(NCORES):
        bs = slice(c * BLOC, (c + 1) * BLOC)
        idx_pack = idx[bs].reshape(BLOC, NT, P).transpose(2, 0, 1).reshape(P, BLOC * NT)
        eq_pack = eq[bs].reshape(BLOC, NT, P).transpose(2, 0, 1).reshape(P, BLOC * NT)
        pvecT_pack = pvec[bs].transpose(2, 0, 1).reshape(HID, BLOC * 2)
        in_maps.append(dict(
            coordT=np.ascontiguousarray(
                coordT[bs].transpose(1, 0, 2).reshape(2, BLOC * N)),
            pvecT=np.ascontiguousarray(pvecT_pack),
            scat_idx=np.ascontiguousarray(idx_pack),
            scat_eq=np.ascontiguousarray(eq_pack),
            **shared,
        ))
    return in_maps


def kernel(**inputs):
    from concourse import bass_utils
    nc = _get_nc()
    in_maps = make_in_maps(inputs)
    res = bass_utils.run_bass_kernel_spmd(nc, in_maps, core_ids=list(range(NCORES)))
    emb = np.concatenate([r["out_emb"] for r in res.results], axis=0)
    adj = np.concatenate([r["out_adj"] for r in res.results], axis=0)
    return emb.astype(np.float32, copy=False), adj.astype(np.int32, copy=False)
